# revision 1
# baseline (speedup 1.0000x reference)
"""GCN (5-layer ColorGNN) Bass kernel for 8 TRN2 NeuronCores.

Strategy (node-sharded, SPMD):
  - Nodes row-sharded across 8 cores (6250/core, padded to 6272 = 49*128).
  - Fixed normalized adjacency A (same for all 5 layers, incl self-loops):
      out[d] = dinv[d] * sum_e(dinv[src_e]*ew_e * T[src_e]) (+ bias via
      sqrt(deg) trick inside PSUM), relu fused in the epilogue.
  - Per layer: dense matmul T_own = H @ W (node-parallel, fp16),
      AllGather T_own -> T_full, then aggregation:
      per 128-dst-node tile: dma_gather message rows from T_full (fp16),
      one-hot selection matrices S (built on DVE: is_equal(iota, dstslot)
      * norm) folded through the PE: psum += S_c^T @ msg_c.
  - Layer 1 aggregates X first (A@X) since F_in=512 < F_out=2048.
  - Host preprocessing: sort edges by (dst tile, src region), pad per
    (tile, region) to the max count over cores so all 8 cores run the
    same program (SPMD) with different index/norm data.
"""

import numpy as np
import concourse.bass as bass
import concourse.mybir as mybir
import concourse.tile as tile

FP16 = mybir.dt.float16
F32 = mybir.dt.float32
I16 = mybir.dt.int16

P = 128
SPLIT = 32768  # int16 index limit boundary for gather regions
SBATCH = 8     # chunks per S-build DVE op batch


# ---------------------------------------------------------------- tile patch
def apply_tile_patch():
    """This walrus build allows only 1 sync-wait per Drain; split the tail
    drain's waits across a chain of drains."""
    import bass_rust

    def _drain_and_barrier_split(self, tick_clock, wait_clock):
        from bass_rust import ScopedClock
        drain_inst = self.nc.sync.drain()
        wait_clock.add_sem_waits(
            drain_inst.ins, ScopedClock({None: tick_clock.global_clock})
        )
        si = drain_inst.ins.sync_info
        waits = list(si.on_wait) if si is not None else []
        if len(waits) > 1:
            si.on_wait = [waits[0]]
            for w in waits[1:]:
                extra = self.nc.sync.drain()
                if extra.ins.sync_info is None:
                    extra.ins.sync_info = bass_rust.SyncInfo(
                        on_wait=[w], on_update=[])
                else:
                    extra.ins.sync_info.on_wait = [w]
        self.nc.all_engine_barrier()
        popped = self.nc._tile_sem_poison_stack.pop()
        assert popped is self._sem_poison
        self.nc.clear_and_free_semaphores(list(self.sems.allocated().values()))
        self.nc.all_engine_barrier()

    tile.TileContext._drain_and_barrier = _drain_and_barrier_split


# ------------------------------------------------------------------- config
class Cfg:
    def __init__(self, n_nodes, n_cores, dims_in):
        # dims_in: [512, 2048, 1024, 512, 128, 64] + final 3
        self.n_nodes = n_nodes
        self.n_cores = n_cores
        self.pcn = n_nodes // n_cores               # real nodes per core
        assert self.pcn * n_cores == n_nodes
        self.npc = ((self.pcn + P - 1) // P) * P    # padded nodes per core
        self.nt = self.npc // P                     # dst tiles per core
        self.npt = self.npc * n_cores               # padded total nodes
        # feature widths (pad last hidden 64->128, final out 3->4)
        d = list(dims_in)
        self.dims_real = d
        self.hid = [d[0], d[1], d[2], d[3], d[4], P]    # hidden widths padded
        self.fout = 4                               # padded final width
        # aggregation widths (width of T_l gathered at layer l)
        self.tw = [self.hid[0], self.hid[2], self.hid[3], self.hid[4], self.hid[5]]
        # regions for int16 gather indexing
        if self.npt > SPLIT:
            assert self.npt - SPLIT <= 32768
            self.regions = [(0, SPLIT), (SPLIT, self.npt)]
        else:
            self.regions = [(0, self.npt)]


# ------------------------------------------------------------- preprocess
def preprocess(x, edge_index, edge_attr, Ws, bs, Wp, bp, cfg: Cfg):
    """Host-side: normalization, edge sharding/sorting/packing, input maps.
    Returns (in_maps, meta). meta holds the compile-time structure."""
    N, C = cfg.n_nodes, cfg.n_cores
    src = np.asarray(edge_index[0], dtype=np.int64)
    dst = np.asarray(edge_index[1], dtype=np.int64)
    ew = np.asarray(edge_attr, dtype=np.float32)
    loop = np.arange(N, dtype=np.int64)
    src2 = np.concatenate([src, loop])
    dst2 = np.concatenate([dst, loop])
    ew2 = np.concatenate([ew, np.ones(N, np.float32)])

    deg = np.bincount(dst2, weights=ew2.astype(np.float64), minlength=N)
    deg = deg.astype(np.float32)
    dinv = np.where(deg > 0, 1.0 / np.sqrt(deg), 0.0).astype(np.float32)
    norm_s = (dinv[src2] * ew2).astype(np.float32)  # dinv[dst] applied later

    gpid = (src2 // cfg.pcn) * cfg.npc + (src2 % cfg.pcn)  # padded global ids

    core_of = dst2 // cfg.pcn
    slot = dst2 - core_of * cfg.pcn          # local slot 0..pcn-1
    tile_of = slot // P
    slot_in = slot % P

    NR = len(cfg.regions)
    region_of = np.zeros(len(src2), np.int64)
    if NR == 2:
        region_of = (gpid >= SPLIT).astype(np.int64)

    # bucket edges per (core, tile, region)
    counts = np.zeros((C, cfg.nt, NR), np.int64)
    np.add.at(counts, (core_of, tile_of, region_of), 1)
    kmax = counts.max(axis=0)                      # [nt, NR]
    K = ((kmax + P - 1) // P) * P                  # padded per-call counts
    K[kmax == 0] = 0

    # order edges by (core, tile, region) via lexsort
    order = np.lexsort((region_of, tile_of, core_of))
    so_gpid = gpid[order]
    so_norm = norm_s[order]
    so_slot = slot_in[order]
    so_core = core_of[order]
    so_tile = tile_of[order]
    so_reg = region_of[order]

    # per-(tile,region) call column bases (idx cols and chunk cols)
    icol = np.zeros((cfg.nt, NR), np.int64)
    cbase = np.zeros((cfg.nt, NR), np.int64)
    ic = cc = 0
    for t in range(cfg.nt):
        for r in range(NR):
            icol[t, r] = ic
            cbase[t, r] = cc
            ic += K[t, r] // 16
            cc += K[t, r] // P
    idxcols, nch = ic, cc

    in_maps = []
    x_np = np.asarray(x, dtype=np.float32)
    # weight packs (shared across cores)
    w_packs, b_rows = [], []
    hid = cfg.hid
    w_list = [np.asarray(w, np.float32) for w in Ws] + [np.asarray(Wp, np.float32)]
    b_list = [np.asarray(b, np.float32) for b in bs] + [np.asarray(bp, np.float32)]
    kdims = [hid[0], hid[1], hid[2], hid[3], hid[4], hid[5]]
    mdims = [hid[1], hid[2], hid[3], hid[4], hid[5], cfg.fout]
    for j in range(6):
        Kd, Md = kdims[j], mdims[j]
        wp = np.zeros((Kd, Md), np.float32)
        wr = w_list[j]
        wp[: wr.shape[0], : wr.shape[1]] = wr
        wp = wp.reshape(Kd // P, P, Md).transpose(1, 0, 2).reshape(P, -1)
        w_packs.append(wp.astype(np.float16))
        br = np.zeros((1, Md), np.float32)
        br[0, : b_list[j].shape[0]] = b_list[j]
        b_rows.append(br.astype(np.float16))

    iota = np.tile(np.arange(P, dtype=np.float16), (P, 1))
    ones1 = np.ones((1, P), np.float16)

    # boundaries of each core's edges in the sorted order
    core_starts = np.searchsorted(so_core, np.arange(C + 1))

    for c in range(C):
        lo, hi = core_starts[c], core_starts[c + 1]
        ct, cr = so_tile[lo:hi], so_reg[lo:hi]
        cg, cn, cs = so_gpid[lo:hi], so_norm[lo:hi], so_slot[lo:hi]
        # per (tile, region) start offsets within this core's slice
        idx16 = np.zeros((16, idxcols), np.int16)
        slotp = np.zeros((P, nch), np.float16)
        normp = np.zeros((P, nch), np.float16)
        pos = 0
        for t in range(cfg.nt):
            for r in range(NR):
                k = K[t, r]
                if k == 0:
                    continue
                n_e = counts[c, t, r]
                seg = slice(pos, pos + n_e)
                assert np.all(ct[seg] == t) and np.all(cr[seg] == r), (c, t, r)
                reg_lo = cfg.regions[r][0]
                arr = np.zeros(k, np.int64)
                arr[:n_e] = cg[seg] - reg_lo
                assert arr.max(initial=0) < 32768
                idx16[:, icol[t, r]: icol[t, r] + k // 16] = (
                    arr.reshape(k // 16, 16).T.astype(np.int16))
                sl = np.zeros(k, np.float32)
                sl[:n_e] = cs[seg]
                nm = np.zeros(k, np.float32)
                nm[:n_e] = cn[seg]
                cb = cbase[t, r]
                slotp[:, cb: cb + k // P] = (
                    sl.reshape(k // P, P).T.astype(np.float16))
                normp[:, cb: cb + k // P] = (
                    nm.reshape(k // P, P).T.astype(np.float16))
                pos += n_e
        assert pos == hi - lo

        # x shard (padded, fp16)
        x16 = np.zeros((cfg.npc, hid[0]), np.float16)
        x16[: cfg.pcn] = x_np[c * cfg.pcn: (c + 1) * cfg.pcn].astype(np.float16)
        # dinv per slot [128, nt], sqrt(deg) row [1, npc]
        dloc = np.zeros(cfg.npc, np.float32)
        dloc[: cfg.pcn] = dinv[c * cfg.pcn: (c + 1) * cfg.pcn]
        dinvp = dloc.reshape(cfg.nt, P).T.copy()
        sq = np.zeros((1, cfg.npc), np.float32)
        sq[0, : cfg.pcn] = np.sqrt(deg[c * cfg.pcn: (c + 1) * cfg.pcn])
        sqd = sq.astype(np.float16)

        m = {
            "x16": x16,
            "idx16": np.tile(idx16, (8, 1)),
            "slotp": slotp,
            "normp": normp,
            "dinvp": dinvp,
            "sqd": sqd,
            "iota": iota,
            "ones1": ones1,
        }
        for j in range(6):
            m[f"w{j}"] = w_packs[j]
            m[f"b{j}"] = b_rows[j]
        in_maps.append(m)

    meta = dict(K=K, icol=icol, cbase=cbase, idxcols=idxcols, nch=nch)
    return in_maps, meta


# ---------------------------------------------------------------- program
def _bc3(ap, ncols, inner=P, mode="col"):
    """3D broadcast APs for batched S-build.
    mode 'col': [128, ncols] -> [[p,128],[1,ncols],[0,inner]] (each col
    replicated across inner); mode 'mat': [128, inner] -> insert [0, ncols]."""
    base = ap.ap
    if mode == "col":
        return bass.AP(ap.tensor, ap.offset, [base[0], [1, ncols], [0, inner]])
    else:
        return bass.AP(ap.tensor, ap.offset, [base[0], [0, ncols], base[1]])


def _3d(ap, ncols, inner=P):
    """[128, ncols*inner] contiguous slice -> [[p,128],[inner,ncols],[1,inner]]"""
    return bass.AP(ap.tensor, ap.offset, [ap.ap[0], [inner, ncols], [1, inner]])


def build_program(cfg: Cfg, meta):
    import concourse.bacc as bacc
    nc = bacc.Bacc("TRN2", num_swdge_queues=4)
    hid, tw = cfg.hid, cfg.tw
    K, icol, cbase = meta["K"], meta["icol"], meta["cbase"]
    idxcols, nch = meta["idxcols"], meta["nch"]
    NR = len(cfg.regions)
    NT = cfg.nt
    rg = [list(range(cfg.n_cores))]

    # ---------------- params
    pr = {}
    pr["x16"] = nc.declare_dram_parameter("x16", [cfg.npc, hid[0]], FP16, isOutput=False)
    pr["idx16"] = nc.declare_dram_parameter("idx16", [P, idxcols], I16, isOutput=False)
    pr["slotp"] = nc.declare_dram_parameter("slotp", [P, nch], FP16, isOutput=False)
    pr["normp"] = nc.declare_dram_parameter("normp", [P, nch], FP16, isOutput=False)
    pr["dinvp"] = nc.declare_dram_parameter("dinvp", [P, NT], F32, isOutput=False)
    pr["sqd"] = nc.declare_dram_parameter("sqd", [1, cfg.npc], FP16, isOutput=False)
    pr["iota"] = nc.declare_dram_parameter("iota", [P, P], FP16, isOutput=False)
    pr["ones1"] = nc.declare_dram_parameter("ones1", [1, P], FP16, isOutput=False)
    kdims = [hid[0], hid[1], hid[2], hid[3], hid[4], hid[5]]
    mdims = [hid[1], hid[2], hid[3], hid[4], hid[5], cfg.fout]
    for j in range(6):
        pr[f"w{j}"] = nc.declare_dram_parameter(
            f"w{j}", [P, (kdims[j] // P) * mdims[j]], FP16, isOutput=False)
        pr[f"b{j}"] = nc.declare_dram_parameter(f"b{j}", [1, mdims[j]], FP16, isOutput=False)
    out_ext = nc.declare_dram_parameter("out", [cfg.pcn, 3], F32, isOutput=True)

    # ---------------- internal DRAM
    xb = nc.dram_tensor("xb", [cfg.npc, hid[0]], FP16)
    TF = [nc.dram_tensor(f"tf{l}", [cfg.npt, tw[l]], FP16, addr_space="Shared")
          for l in range(5)]
    town = [None] + [nc.dram_tensor(f"town{l}", [cfg.npc, tw[l]], FP16)
                     for l in range(1, 5)]
    # dense outputs: G1 (agg of x), H1..H5
    G1 = nc.dram_tensor("g1", [cfg.npc, tw[0]], FP16)
    Hs = [nc.dram_tensor(f"h{j}", [cfg.npc, hid[j + 1]], FP16) for j in range(5)]

    with tile.TileContext(nc) as tc:
        import contextlib
        with contextlib.ExitStack() as ctx:
            cpool = ctx.enter_context(tc.tile_pool(name="const", bufs=1))
            msgp = ctx.enter_context(tc.tile_pool(name="msg", bufs=3))
            spool = ctx.enter_context(tc.tile_pool(name="sb", bufs=2))
            pp = ctx.enter_context(tc.tile_pool(name="ps", bufs=2, space="PSUM"))
            hp = ctx.enter_context(tc.tile_pool(name="hout", bufs=3))
            wp_ = ctx.enter_context(tc.tile_pool(name="wts", bufs=1))
            htp = ctx.enter_context(tc.tile_pool(name="ht", bufs=24))

            # ---- resident constants
            idx_sb = cpool.tile([P, idxcols], I16)
            nc.sync.dma_start(out=idx_sb[:], in_=pr["idx16"][:])
            slot_sb = cpool.tile([P, nch], FP16)
            nc.sync.dma_start(out=slot_sb[:], in_=pr["slotp"][:])
            norm_sb = cpool.tile([P, nch], FP16)
            nc.sync.dma_start(out=norm_sb[:], in_=pr["normp"][:])
            dinv_sb = cpool.tile([P, NT], F32)
            nc.sync.dma_start(out=dinv_sb[:], in_=pr["dinvp"][:])
            sqd_sb = cpool.tile([1, cfg.npc], FP16)
            nc.sync.dma_start(out=sqd_sb[:], in_=pr["sqd"][:])
            iota_sb = cpool.tile([P, P], FP16)
            nc.sync.dma_start(out=iota_sb[:], in_=pr["iota"][:])
            ones_sb = cpool.tile([1, P], FP16)
            nc.sync.dma_start(out=ones_sb[:], in_=pr["ones1"][:])
            brow_sb = []
            for j in range(6):
                b_ = cpool.tile([1, mdims[j]], FP16, tag=f"br{j}")
                nc.sync.dma_start(out=b_[:], in_=pr[f"b{j}"][:])
                brow_sb.append(b_)

            GMAX = 8  # max chunks per gather call (ucode caps dma_gather at 1024 idxs)
            qn = [0]  # round-robin SWDGE queue

            # ---- aggregation phase for layer l (0-based): T_full -> dst
            def agg(l, dst_dram, with_bias_relu, bias_idx):
                W = tw[l]
                FC = min(W, 512)
                nfp = W // FC
                for t in range(NT):
                    ct = int(K[t].sum() // P)
                    # gather segments: (region, n_idx, idx_col, chunk_off)
                    segs = []
                    for r in range(NR):
                        k, pos = int(K[t, r]), 0
                        while pos < k:
                            ks = min(GMAX * P, k - pos)
                            segs.append((r, ks, int(icol[t, r]) + pos // 16,
                                         int(cbase[t, r] - cbase[t, 0]) + pos // P))
                            pos += ks
                    # build S for all chunks of this tile
                    s_t = spool.tile([P, max(ct, 1) * P], FP16, tag="s")
                    for b0 in range(0, ct, SBATCH):
                        nb = min(SBATCH, ct - b0)
                        cb0 = int(cbase[t, 0]) + b0
                        o3 = _3d(s_t[:, b0 * P:(b0 + nb) * P], nb)
                        nc.vector.tensor_tensor(
                            out=o3,
                            in0=_bc3(slot_sb[:, cb0:cb0 + nb], nb, mode="col"),
                            in1=_bc3(iota_sb[:], nb, mode="mat"),
                            op=mybir.AluOpType.is_equal)
                        nc.vector.tensor_tensor(
                            out=o3, in0=o3,
                            in1=_bc3(norm_sb[:, cb0:cb0 + nb], nb, mode="col"),
                            op=mybir.AluOpType.mult)
                    ps = pp.tile([P, 2048], F32, tag="ps")
                    for fp in range(nfp):
                        nmm = 0
                        for (r, ks, ic, cb) in segs:
                            reg_lo, reg_hi = cfg.regions[r]
                            src_ap = TF[l][reg_lo:reg_hi, fp * FC:(fp + 1) * FC]
                            msg = msgp.tile([P, GMAX * FC], FP16, tag="msg")
                            nc.gpsimd.dma_gather(
                                out_ap=_3d(msg[:, : (ks // P) * FC],
                                           ks // P, inner=FC),
                                in_ap=src_ap,
                                idxs_ap=idx_sb[:, ic: ic + ks // 16],
                                num_idxs=ks,
                                num_idxs_reg=ks,
                                elem_size=FC,
                                elem_step=W,
                                queue_num=qn[0],
                            )
                            qn[0] = (qn[0] + 1) % 4
                            for ci in range(ks // P):
                                nmm += 1
                                nc.tensor.matmul(
                                    out=ps[:, fp * FC:(fp + 1) * FC],
                                    lhsT=s_t[:, (cb + ci) * P:(cb + ci + 1) * P],
                                    rhs=msg[:, ci * FC:(ci + 1) * FC],
                                    start=(nmm == 1),
                                    stop=(not with_bias_relu and nmm == ct))
                        # bias matmul: psum += sqrt(deg)[:,None] @ b[None,:]
                        if with_bias_relu:
                            nc.tensor.matmul(
                                out=ps[:, fp * FC:(fp + 1) * FC],
                                lhsT=sqd_sb[0:1, t * P:(t + 1) * P],
                                rhs=brow_sb[bias_idx][0:1, fp * FC:(fp + 1) * FC],
                                start=(ct == 0), stop=True)
                        elif ct == 0:
                            nc.vector.memset(ps[:, fp * FC:(fp + 1) * FC], 0.0)
                    h_sb = hp.tile([P, W], FP16, tag="hout")
                    nc.scalar.activation(
                        out=h_sb[:], in_=ps[:, :W],
                        func=(mybir.ActivationFunctionType.Relu if with_bias_relu
                              else mybir.ActivationFunctionType.Copy),
                        scale=dinv_sb[:, t:t + 1])
                    nc.sync.dma_start(
                        out=dst_dram[t * P:(t + 1) * P, :], in_=h_sb[:])

            # ---- dense phase j (0-based): in_dram [npc,K] @ w_j -> out
            def dense(j, in_dram, out_dram, bias_relu, final=False):
                Kd, Md = kdims[j], mdims[j]
                nk = Kd // P
                w_sb = wp_.tile([P, nk * Md], FP16, tag="w")
                nc.sync.dma_start(out=w_sb[:], in_=pr[f"w{j}"][:])
                for t in range(NT):
                    ps = pp.tile([P, 2048], F32, tag="ps")
                    hts = []
                    for k in range(nk):
                        ht = htp.tile([P, P], FP16, tag="ht")
                        nc.sync.dma_start(
                            out=ht[:],
                            in_=in_dram[t * P:(t + 1) * P, k * P:(k + 1) * P],
                            transpose=True)
                        hts.append(ht)
                    has_bias = bias_relu or final
                    for k in range(nk):
                        for m0 in range(0, Md, 512):
                            m1 = min(m0 + 512, Md)
                            nc.tensor.matmul(
                                out=ps[:, m0:m1],
                                lhsT=hts[k][:],
                                rhs=w_sb[:, k * Md + m0: k * Md + m1],
                                start=(k == 0),
                                stop=(k == nk - 1 and not has_bias))
                    if bias_relu or final:
                        for m0 in range(0, Md, 512):
                            m1 = min(m0 + 512, Md)
                            nc.tensor.matmul(
                                out=ps[:, m0:m1],
                                lhsT=ones_sb[0:1, :],
                                rhs=brow_sb[j][0:1, m0:m1],
                                start=False, stop=True)
                    if final:
                        o_sb = hp.tile([P, Md], F32, tag="fout")
                        nc.vector.tensor_copy(out=o_sb[:], in_=ps[:, :Md])
                        r0 = t * P
                        r1 = min((t + 1) * P, cfg.pcn)
                        if r1 > r0:
                            nc.sync.dma_start(
                                out=out_dram[r0:r1, :],
                                in_=o_sb[: r1 - r0, :3])
                    else:
                        h_sb = hp.tile([P, Md], FP16, tag="hout")
                        if bias_relu:
                            nc.scalar.activation(
                                out=h_sb[:], in_=ps[:, :Md],
                                func=mybir.ActivationFunctionType.Relu)
                        else:
                            nc.scalar.copy(out=h_sb[:], in_=ps[:, :Md])
                        nc.sync.dma_start(
                            out=out_dram[t * P:(t + 1) * P, :], in_=h_sb[:])

            # ---------------- the network
            nc.sync.dma_start(out=xb[:], in_=pr["x16"][:])
            nc.gpsimd.collective_compute(
                "AllGather", mybir.AluOpType.bypass, replica_groups=rg,
                ins=[xb[:]], outs=[TF[0][:]])
            agg(0, G1, with_bias_relu=False, bias_idx=None)       # A@X
            dense(0, G1, Hs[0], bias_relu=True)                   # H1
            for l in range(1, 5):
                dense(l, Hs[l - 1], town[l], bias_relu=False)     # T_own
                nc.gpsimd.collective_compute(
                    "AllGather", mybir.AluOpType.bypass, replica_groups=rg,
                    ins=[town[l][:]], outs=[TF[l][:]])
                agg(l, Hs[l], with_bias_relu=True, bias_idx=l)    # H_{l+1}
            dense(5, Hs[4], out_ext, bias_relu=False, final=True)

    nc.finalize()
    return nc


# ------------------------------------------------------------------ driver
def run_numpy_reference(x, edge_index, edge_attr, Ws, bs, Wp, bp):
    """Mirror of reference.py in numpy (float32)."""
    N = x.shape[0]
    src, dst = np.asarray(edge_index[0]), np.asarray(edge_index[1])
    ew = np.asarray(edge_attr, np.float32)
    loop = np.arange(N)
    src2 = np.concatenate([src, loop])
    dst2 = np.concatenate([dst, loop])
    ew2 = np.concatenate([ew, np.ones(N, np.float32)])
    deg = np.bincount(dst2, weights=ew2, minlength=N).astype(np.float32)
    dinv = np.where(deg > 0, 1 / np.sqrt(deg), 0).astype(np.float32)
    norm = dinv[src2] * ew2 * dinv[dst2]

    def conv(h, W, b):
        hw = h @ W
        msg = hw[src2] * norm[:, None]
        out = np.zeros((N, W.shape[1]), np.float32)
        np.add.at(out, dst2, msg)
        return out + b

    h = np.asarray(x, np.float32)
    for W, b in zip(Ws, bs):
        h = np.maximum(conv(h, W, b), 0)
    return h @ Wp + bp


# ===================================================================
# Harness entry point: kernel(**inputs) -> np.ndarray [50000, 3] f32
# ===================================================================
_CACHE = {}


def kernel(x, edge_index, edge_attr, W1, b1, W2, b2, W3, b3, W4, b4, W5, b5,
           Wp, bp):
    apply_tile_patch()
    import os
    from concourse.bass_utils import run_bass_kernel_spmd

    cfg = Cfg(50000, 8, [512, 2048, 1024, 512, 128, 64])
    Ws = [W1, W2, W3, W4, W5]
    bs = [b1, b2, b3, b4, b5]
    in_maps, meta = preprocess(x, edge_index, edge_attr, Ws, bs, Wp, bp, cfg)

    key = (meta["K"].tobytes(), meta["nch"], meta["idxcols"])
    nc = _CACHE.get(key)
    if nc is None:
        nc = build_program(cfg, meta)
        _CACHE[key] = nc

    res = run_bass_kernel_spmd(
        nc, in_maps, core_ids=list(range(cfg.n_cores)),
        trace=bool(int(os.environ.get("TRACE", "0"))))
    if res.exec_time_ns:
        print(f"HW exec time: {res.exec_time_ns} ns")
    out = np.concatenate(
        [res.results[c]["out"] for c in range(cfg.n_cores)], axis=0)
    return np.ascontiguousarray(out.astype(np.float32))

